# revision 3
# baseline (speedup 1.0000x reference)
"""Gated GCN layer (DDI message passing) on 8 Trainium2 NeuronCores — v3.

Data-parallel over batch B=256 -> 32 sentences/core. Host builds dense
per-sentence adjacency M (and M^T); aggregations are dense matmuls.

v3 vs baseline:
  - Aggregation results stay in PSUM (ap, bufs=2); the gating epilogue
    reads them directly — the baseline's agg PSUM->SBUF copy is gone.
  - The loop group of the main GEMM also stays in PSUM (zloop, bufs=2):
    its gate sigmoid and t1 = loopZ*sigma_loop read PSUM directly, so
    the main Z copy shrinks to the in/out groups (728 cols vs 1092).
  - zmain (in/out groups) single-buffered: its copy drains during the
    next agg+main bursts. PSUM: 2 + 2 + 4 = 8 banks.
  - Epilogue ops only on scalar (sigmoids, relu) and vector (t1 mul,
    two fused scalar_tensor_tensor) — gpsimd elementwise is ~60x
    slower than DVE and is used for nothing; sync/gpsimd drive DMA.

SBUF zs layout per sentence: [128, 2 blocks x 2 groups x 364] bf16,
block0 = node rows 0:128, block1 = rows 128:200 (partitions 0:72).
Group: 360 feats + gate col at 360. Groups: in | out.
"""
import sys

if "/opt/trn_rl_repo" not in sys.path:
    sys.path.insert(0, "/opt/trn_rl_repo")

from contextlib import ExitStack

import ml_dtypes
import numpy as np

B, NN, EE, DIN, DOUT = 256, 200, 400, 360, 360
NCORES = 8
SPC = B // NCORES          # 32 sentences per core
ROWS = SPC * NN            # 6400 rows per core
KA = DIN + 1               # 361: augmented contraction (ones row for bias)
GP = 364                   # group stride in SBUF (4B aligned for bf16)
B2 = 2 * GP                # block stride in zs (in/out groups only)
PSG = 512                  # PSUM bank stride (fp32)
KCH = [(0, 121), (121, 241), (241, 361)]   # K chunks <= 128
NBF16 = np.dtype(ml_dtypes.bfloat16)

_compiled = None


def _build():
    import concourse.bacc as bacc
    import concourse.mybir as mybir
    from concourse.tile import TileContext

    BF16 = mybir.dt.bfloat16
    F32 = mybir.dt.float32
    AF = mybir.ActivationFunctionType
    OP = mybir.AluOpType

    nc = bacc.Bacc(None, target_bir_lowering=False)
    xt_d = nc.dram_tensor("xt", [KA, ROWS], BF16, kind="ExternalInput")
    wt_d = nc.dram_tensor("wt", [KA, 3 * GP], BF16, kind="ExternalInput")
    mfb_d = nc.dram_tensor("mfb", [SPC, 128, 800], BF16, kind="ExternalInput")
    out_d = nc.dram_tensor("out", [SPC, 128, 2 * DOUT], BF16, kind="ExternalOutput")

    with TileContext(nc) as tc, ExitStack() as ctx:
        cpool = ctx.enter_context(tc.tile_pool(name="cpool", bufs=1))
        mpool = ctx.enter_context(tc.tile_pool(name="mpool", bufs=32))
        spool = ctx.enter_context(tc.tile_pool(name="spool", bufs=3))
        tpool = ctx.enter_context(tc.tile_pool(name="tpool", bufs=6))
        zmpool = ctx.enter_context(tc.tile_pool(name="zmpool", bufs=1, space="PSUM"))
        zlpool = ctx.enter_context(tc.tile_pool(name="zlpool", bufs=2, space="PSUM"))
        apool = ctx.enter_context(tc.tile_pool(name="apool", bufs=2, space="PSUM"))

        # ---- resident inputs: smallest pieces first so matmul 1 can
        # start ASAP; each kc chunk rides its own engine queue ----
        eng3 = [nc.scalar, nc.gpsimd, nc.sync]
        wt_tiles = [cpool.tile([k1 - k0, 3 * GP], BF16, name=f"wt{kc}")
                    for kc, (k0, k1) in enumerate(KCH)]
        xt_tiles = [cpool.tile([k1 - k0, ROWS], BF16, name=f"xt{kc}")
                    for kc, (k0, k1) in enumerate(KCH)]

        state = {}       # sentence -> dict(mfb, zs, ot, t1 per mt)

        def get_state(s):
            if s not in state:
                mfb_t = mpool.tile([128, 800], BF16, tag="mfb", name=f"mfb{s}")
                nc.sync.dma_start(out=mfb_t, in_=mfb_d[s])
                state[s] = {"mfb": mfb_t}
            return state[s]

        # warmup buffer first: memset rides ahead of gpsimd's DMA queue
        wup = cpool.tile([128, 512], BF16, name="wup")
        nc.gpsimd.memset(wup, 0.0)

        for kc, (k0, k1) in enumerate(KCH):
            eng3[kc].dma_start(out=xt_tiles[kc][:, 0:128], in_=xt_d[k0:k1, 0:128])
        for kc, (k0, k1) in enumerate(KCH):
            for g in range(3):
                eng3[g].dma_start(out=wt_tiles[kc][:, g * GP:(g + 1) * GP],
                                  in_=wt_d[k0:k1, g * GP:(g + 1) * GP])
        for kc, (k0, k1) in enumerate(KCH):
            eng3[kc].dma_start(out=xt_tiles[kc][:, 128:400], in_=xt_d[k0:k1, 128:400])
        for s in range(SPC):
            get_state(s)

        # rest of xt all on gpsimd: it is otherwise idle until the first
        # output DMAs (~20us in), and keeping sync free keeps mfb flowing
        for a, b in [(400, 1200), (1200, 2400), (2400, 3800),
                     (3800, 5100), (5100, ROWS)]:
            for kc, (k0, k1) in enumerate(KCH):
                nc.gpsimd.dma_start(out=xt_tiles[kc][:, a:b], in_=xt_d[k0:k1, a:b])

        # ---- PE warmup: hold the activity monitor at full clock while
        # the first input DMAs land (junk matmuls on zeroed SBUF) ----
        wps = apool.tile([128, 2 * PSG], F32, tag="ap", name="wps")
        for i in range(20):
            nc.tensor.matmul(wps[:, 0:512], lhsT=wup[:, 0:128],
                             rhs=wup[:, 0:512], start=True, stop=True)

        def emit_main(s, mt):
            rows = 128 if mt == 0 else 72
            c0 = s * NN + mt * 128
            st = get_state(s)
            if "zs" not in st:
                st["zs"] = spool.tile([128, 2 * B2], BF16, tag="zs", name=f"zs{s}")
                st["ot"] = tpool.tile([128, 2 * DOUT], BF16, tag="ot", name=f"ot{s}")
            zm = zmpool.tile([128, 2 * PSG], F32, tag="zm", name=f"zm{s}_{mt}")
            zl = zlpool.tile([128, PSG], F32, tag="zl", name=f"zl{s}_{mt}")
            for kc, (k0, k1) in enumerate(KCH):
                lhs = xt_tiles[kc][:, c0:c0 + rows]
                st_, sp_ = (kc == 0), (kc == 2)
                nc.tensor.matmul(zm[0:rows, 0:361], lhsT=lhs,
                                 rhs=wt_tiles[kc][:, 0:361], start=st_, stop=sp_)
                nc.tensor.matmul(zm[0:rows, PSG:PSG + 361], lhsT=lhs,
                                 rhs=wt_tiles[kc][:, GP:GP + 361], start=st_, stop=sp_)
                nc.tensor.matmul(zl[0:rows, 0:361], lhsT=lhs,
                                 rhs=wt_tiles[kc][:, 2 * GP:2 * GP + 361],
                                 start=st_, stop=sp_)
            # loop-group epilogue pieces (early, straight from PSUM)
            sgl = tpool.tile([128, 2], F32, tag="sgl", name=f"sgl{s}_{mt}")
            nc.scalar.activation(sgl[0:rows, 0:1], zl[0:rows, 360:361], AF.Sigmoid)
            t1 = tpool.tile([128, DOUT], BF16, tag="t1", name=f"t1{s}_{mt}")
            nc.vector.tensor_scalar_mul(t1[0:rows], zl[0:rows, 0:360], sgl[0:rows, 0:1])
            # main Z copy: in/out groups -> zs block
            zs = st["zs"]
            src = zm[0:rows, :].rearrange("p (g c) -> p g c", g=2)[:, :, 0:GP]
            dst = zs[0:rows, mt * B2:(mt + 1) * B2].rearrange("p (g c) -> p g c", g=2)
            nc.scalar.copy(dst, src)
            st[("t1", mt)] = t1

        def emit_agg(s, mt):
            st = state[s]
            mfb_t, zs, ot = st["mfb"], st["zs"], st["ot"]
            t1 = st.pop(("t1", mt))
            rows = 128 if mt == 0 else 72
            d0 = mt * 128
            ap_ = apool.tile([128, 2 * PSG], F32, tag="ap", name=f"ap{s}_{mt}")
            nc.tensor.matmul(ap_[0:rows, 0:361], lhsT=mfb_t[0:128, d0:d0 + rows],
                             rhs=zs[0:128, 0:361], start=True, stop=False)
            nc.tensor.matmul(ap_[0:rows, 0:361], lhsT=mfb_t[0:72, 200 + d0:200 + d0 + rows],
                             rhs=zs[0:72, B2:B2 + 361], start=False, stop=True)
            nc.tensor.matmul(ap_[0:rows, PSG:PSG + 361], lhsT=mfb_t[0:128, 400 + d0:400 + d0 + rows],
                             rhs=zs[0:128, GP:GP + 361], start=True, stop=False)
            nc.tensor.matmul(ap_[0:rows, PSG:PSG + 361], lhsT=mfb_t[0:72, 600 + d0:600 + d0 + rows],
                             rhs=zs[0:72, B2 + GP:B2 + GP + 361], start=False, stop=True)

            sgt = tpool.tile([128, 2], F32, tag="sgt", name=f"sg{s}_{mt}")
            nc.scalar.activation(
                sgt[0:rows, 0:2],
                ap_[0:rows, :].rearrange("p (g c) -> p g c", g=2)[:, :, 360],
                AF.Sigmoid)
            if s == SPC - 1 and mt == 1:
                st["tail"] = (ap_, sgt, t1)
                return
            t2 = tpool.tile([128, DOUT], BF16, tag="t2", name=f"t2{s}_{mt}")
            t3 = tpool.tile([128, DOUT], BF16, tag="t3", name=f"t3{s}_{mt}")
            nc.vector.scalar_tensor_tensor(
                out=t2[0:rows], in0=ap_[0:rows, 0:360],
                scalar=sgt[0:rows, 0:1], in1=t1[0:rows],
                op0=OP.mult, op1=OP.add)
            nc.vector.scalar_tensor_tensor(
                out=t3[0:rows], in0=ap_[0:rows, PSG:PSG + 360],
                scalar=sgt[0:rows, 1:2], in1=t2[0:rows],
                op0=OP.mult, op1=OP.add)
            st[("t3", mt)] = t3

        def emit_relu(s, mt):
            # deferred one pipeline step: keeps Z copies (PE-critical) at the
            # head of scalar's in-order queue, relu/DMA are latency-tolerant
            st = state[s]
            rows = 128 if mt == 0 else 72
            ot = st["ot"]
            t3 = st.pop(("t3", mt))
            nc.scalar.activation(
                ot[0:rows, mt * DOUT:(mt + 1) * DOUT], t3[0:rows], AF.Relu)
            if s == SPC - 1:
                nc.sync.dma_start(
                    out=out_d[s, 0:rows, mt * DOUT:(mt + 1) * DOUT],
                    in_=ot[0:rows, mt * DOUT:(mt + 1) * DOUT])
                if mt == 1:
                    state.pop(s)
            elif mt == 1:
                nc.gpsimd.dma_start(out=out_d[s], in_=ot)
                state.pop(s)

        def emit_tail(s):
            # last unit: halved epilogue chain pipelines STT/relu/DMA
            st = state[s]
            ap_, sgt, t1 = st.pop("tail")
            ot = st["ot"]
            rows, H = 72, 180
            t2 = tpool.tile([128, DOUT], BF16, tag="t2", name=f"t2{s}_tl")
            t3 = tpool.tile([128, DOUT], BF16, tag="t3", name=f"t3{s}_tl")
            for h in range(2):
                cs = slice(h * H, (h + 1) * H)
                nc.vector.scalar_tensor_tensor(
                    out=t2[0:rows, cs], in0=ap_[0:rows, h * H:h * H + H],
                    scalar=sgt[0:rows, 0:1], in1=t1[0:rows, cs],
                    op0=OP.mult, op1=OP.add)
                nc.vector.scalar_tensor_tensor(
                    out=t3[0:rows, cs], in0=ap_[0:rows, PSG + h * H:PSG + h * H + H],
                    scalar=sgt[0:rows, 1:2], in1=t2[0:rows, cs],
                    op0=OP.mult, op1=OP.add)
                nc.scalar.activation(
                    ot[0:rows, DOUT + h * H:DOUT + (h + 1) * H],
                    t3[0:rows, cs], AF.Relu)
                nc.sync.dma_start(
                    out=out_d[s, 0:rows, DOUT + h * H:DOUT + (h + 1) * H],
                    in_=ot[0:rows, DOUT + h * H:DOUT + (h + 1) * H])
            state.pop(s)

        # ---- software pipeline: aggregation lags main by one sentence,
        # relu/output-DMA lag one more step ----
        for s in range(SPC):
            emit_main(s, 0)
            if s > 0:
                emit_agg(s - 1, 0)
            if s > 1:
                emit_relu(s - 2, 1)
            emit_main(s, 1)
            if s == SPC - 1:
                emit_agg(s, 0)          # shorten the drain: last sentence's
                emit_agg(s - 1, 1)      # mt0 slots in before the stragglers
                emit_relu(s - 1, 0)
                emit_agg(s, 1)
                emit_relu(s - 1, 1)
                emit_relu(s, 0)
                emit_tail(s)
            elif s > 0:
                emit_agg(s - 1, 1)
                emit_relu(s - 1, 0)

    nc.compile()
    return nc


def _get_compiled():
    global _compiled
    if _compiled is None:
        _compiled = _build()
    return _compiled


def kernel(gcn_in, adj_ind, adj_data, w_in, b_in, w_out, b_out, w_loop,
           w_gin, b_gin, w_gout, b_gout, w_gloop):
    from concourse.bass_utils import run_bass_kernel_spmd

    x = np.asarray(gcn_in, np.float32)           # [B, N, DIN]
    idx = np.asarray(adj_ind)[0]                 # [B, E, 2] int
    dat = np.asarray(adj_data, np.float32)[0]    # [B, E]

    # fused weight matrix with bias row
    wt = np.zeros((KA, 3 * GP), np.float32)
    for g, (w, gw, bias, gb) in enumerate([
        (w_in, w_gin, b_in, b_gin),
        (w_out, w_gout, b_out, b_gout),
        (w_loop, w_gloop, None, None),
    ]):
        wt[0:DIN, g * GP:g * GP + DOUT] = np.asarray(w, np.float32)
        wt[0:DIN, g * GP + DOUT] = np.asarray(gw, np.float32)[:, 0]
        if bias is not None:
            wt[DIN, g * GP:g * GP + DOUT] = np.asarray(bias, np.float32)[0]
            wt[DIN, g * GP + DOUT] = np.asarray(gb, np.float32)[0]
    wt = wt.astype(NBF16)

    # dense per-sentence adjacency matrices
    M = np.zeros((B, NN, NN), np.float32)
    bi = np.broadcast_to(np.arange(B)[:, None], idx.shape[:2])
    np.add.at(M, (bi, idx[:, :, 0].astype(np.int64), idx[:, :, 1].astype(np.int64)), dat)

    def chunked(mm):      # [SPC,200,200] -> [SPC,128,400]: two 128-row src chunks
        out = np.zeros((SPC, 128, 2 * NN), np.float32)
        out[:, :, 0:NN] = mm[:, 0:128, :]
        out[:, 0:72, NN:2 * NN] = mm[:, 128:200, :]
        return out

    nc = _get_compiled()
    in_maps = []
    for c in range(NCORES):
        xc = x[c * SPC:(c + 1) * SPC].reshape(ROWS, DIN)
        xt = np.empty((KA, ROWS), np.float32)
        xt[0:DIN] = xc.T
        xt[DIN] = 1.0
        mc = M[c * SPC:(c + 1) * SPC]
        mfb = np.concatenate(
            [chunked(mc), chunked(np.ascontiguousarray(mc.transpose(0, 2, 1)))],
            axis=2)
        in_maps.append({
            "xt": np.ascontiguousarray(xt).astype(NBF16),
            "wt": wt,
            "mfb": np.ascontiguousarray(mfb).astype(NBF16),
        })

    res = run_bass_kernel_spmd(nc, in_maps, core_ids=list(range(NCORES)))
    kernel.last_results = res
    out = np.empty((B, NN, DOUT), np.float32)
    for c in range(NCORES):
        oc = res.results[c]["out"].astype(np.float32)   # [SPC,128,720]
        oc_s = out[c * SPC:(c + 1) * SPC]               # [SPC,200,360]
        oc_s[:, 0:128, :] = oc[:, :, 0:DOUT]
        oc_s[:, 128:200, :] = oc[:, 0:72, DOUT:2 * DOUT]
    return out


# revision 4
# speedup vs baseline: 1.0065x; 1.0065x over previous
"""Gated GCN layer (DDI message passing) on 8 Trainium2 NeuronCores — v3.

Data-parallel over batch B=256 -> 32 sentences/core. Host builds dense
per-sentence adjacency M (and M^T); aggregations are dense matmuls.

v3 vs baseline:
  - Aggregation results stay in PSUM (ap, bufs=2); the gating epilogue
    reads them directly — the baseline's agg PSUM->SBUF copy is gone.
  - The loop group of the main GEMM also stays in PSUM (zloop, bufs=2):
    its gate sigmoid and t1 = loopZ*sigma_loop read PSUM directly, so
    the main Z copy shrinks to the in/out groups (728 cols vs 1092).
  - zmain (in/out groups) single-buffered: its copy drains during the
    next agg+main bursts. PSUM: 2 + 2 + 4 = 8 banks.
  - Epilogue ops only on scalar (sigmoids, relu) and vector (t1 mul,
    two fused scalar_tensor_tensor) — gpsimd elementwise is ~60x
    slower than DVE and is used for nothing; sync/gpsimd drive DMA.
  - relu + output DMA deferred one pipeline step so the PE-critical Z
    copies stay at the head of scalar's in-order queue.
  - PE warmup matmuls on zeroed SBUF hold the HAM activity monitor at
    full clock (K=8/8) while the cold-start DMAs land (~13us); without
    it the first ~20us of real matmuls run at half rate.
  - All 32 adjacency tiles prefetched up front (fits SBUF easily).

SBUF zs layout per sentence: [128, 2 blocks x 2 groups x 364] bf16,
block0 = node rows 0:128, block1 = rows 128:200 (partitions 0:72).
Group: 360 feats + gate col at 360. Groups: in | out.
"""
import sys

if "/opt/trn_rl_repo" not in sys.path:
    sys.path.insert(0, "/opt/trn_rl_repo")

from contextlib import ExitStack

import ml_dtypes
import numpy as np

B, NN, EE, DIN, DOUT = 256, 200, 400, 360, 360
NCORES = 8
SPC = B // NCORES          # 32 sentences per core
ROWS = SPC * NN            # 6400 rows per core
KA = DIN + 1               # 361: augmented contraction (ones row for bias)
GP = 364                   # group stride in SBUF (4B aligned for bf16)
B2 = 2 * GP                # block stride in zs (in/out groups only)
PSG = 512                  # PSUM bank stride (fp32)
KCH = [(0, 121), (121, 241), (241, 361)]   # K chunks <= 128
NBF16 = np.dtype(ml_dtypes.bfloat16)

_compiled = None


def _build():
    import concourse.bacc as bacc
    import concourse.mybir as mybir
    from concourse.tile import TileContext

    BF16 = mybir.dt.bfloat16
    F32 = mybir.dt.float32
    AF = mybir.ActivationFunctionType
    OP = mybir.AluOpType

    nc = bacc.Bacc(None, target_bir_lowering=False)
    xt_d = nc.dram_tensor("xt", [KA, ROWS], BF16, kind="ExternalInput")
    wt_d = nc.dram_tensor("wt", [KA, 3 * GP], BF16, kind="ExternalInput")
    mfb_d = nc.dram_tensor("mfb", [SPC, 128, 800], BF16, kind="ExternalInput")
    out_d = nc.dram_tensor("out", [SPC, 128, 2 * DOUT], BF16, kind="ExternalOutput")

    with TileContext(nc) as tc, ExitStack() as ctx:
        cpool = ctx.enter_context(tc.tile_pool(name="cpool", bufs=1))
        mpool = ctx.enter_context(tc.tile_pool(name="mpool", bufs=32))
        spool = ctx.enter_context(tc.tile_pool(name="spool", bufs=3))
        tpool = ctx.enter_context(tc.tile_pool(name="tpool", bufs=6))
        zmpool = ctx.enter_context(tc.tile_pool(name="zmpool", bufs=1, space="PSUM"))
        zlpool = ctx.enter_context(tc.tile_pool(name="zlpool", bufs=2, space="PSUM"))
        apool = ctx.enter_context(tc.tile_pool(name="apool", bufs=2, space="PSUM"))

        # ---- resident inputs: smallest pieces first so matmul 1 can
        # start ASAP; each kc chunk rides its own engine queue ----
        eng3 = [nc.scalar, nc.gpsimd, nc.sync]
        wt_tiles = [cpool.tile([k1 - k0, 3 * GP], BF16, name=f"wt{kc}")
                    for kc, (k0, k1) in enumerate(KCH)]
        xt_tiles = [cpool.tile([k1 - k0, ROWS], BF16, name=f"xt{kc}")
                    for kc, (k0, k1) in enumerate(KCH)]

        state = {}       # sentence -> dict(mfb, zs, ot, t1 per mt)

        def get_state(s):
            if s not in state:
                mfb_t = mpool.tile([128, 800], BF16, tag="mfb", name=f"mfb{s}")
                nc.sync.dma_start(out=mfb_t, in_=mfb_d[s])
                state[s] = {"mfb": mfb_t}
            return state[s]

        # warmup buffer first: memset rides ahead of gpsimd's DMA queue
        wup = cpool.tile([128, 512], BF16, name="wup")
        nc.gpsimd.memset(wup, 0.0)

        for kc, (k0, k1) in enumerate(KCH):
            eng3[kc].dma_start(out=xt_tiles[kc][:, 0:128], in_=xt_d[k0:k1, 0:128])
        for kc, (k0, k1) in enumerate(KCH):
            for g in range(3):
                eng3[g].dma_start(out=wt_tiles[kc][:, g * GP:(g + 1) * GP],
                                  in_=wt_d[k0:k1, g * GP:(g + 1) * GP])
        for kc, (k0, k1) in enumerate(KCH):
            eng3[kc].dma_start(out=xt_tiles[kc][:, 128:400], in_=xt_d[k0:k1, 128:400])
        for s in range(SPC):
            get_state(s)

        # rest of xt all on gpsimd: it is otherwise idle until the first
        # output DMAs (~20us in), and keeping sync free keeps mfb flowing
        for a, b in [(400, 1200), (1200, 2400), (2400, 3800),
                     (3800, 5100), (5100, ROWS)]:
            for kc, (k0, k1) in enumerate(KCH):
                nc.gpsimd.dma_start(out=xt_tiles[kc][:, a:b], in_=xt_d[k0:k1, a:b])

        # ---- PE warmup: hold the activity monitor at full clock while
        # the first input DMAs land (junk matmuls on zeroed SBUF) ----
        wps = apool.tile([128, 2 * PSG], F32, tag="ap", name="wps")
        for i in range(20):
            nc.tensor.matmul(wps[:, 0:512], lhsT=wup[:, 0:128],
                             rhs=wup[:, 0:512], start=True, stop=True)

        def emit_main(s, mt):
            rows = 128 if mt == 0 else 72
            c0 = s * NN + mt * 128
            st = get_state(s)
            if "zs" not in st:
                st["zs"] = spool.tile([128, 2 * B2], BF16, tag="zs", name=f"zs{s}")
                st["ot"] = tpool.tile([128, 2 * DOUT], BF16, tag="ot", name=f"ot{s}")
            zm = zmpool.tile([128, 2 * PSG], F32, tag="zm", name=f"zm{s}_{mt}")
            zl = zlpool.tile([128, PSG], F32, tag="zl", name=f"zl{s}_{mt}")
            for kc, (k0, k1) in enumerate(KCH):
                lhs = xt_tiles[kc][:, c0:c0 + rows]
                st_, sp_ = (kc == 0), (kc == 2)
                nc.tensor.matmul(zm[0:rows, 0:361], lhsT=lhs,
                                 rhs=wt_tiles[kc][:, 0:361], start=st_, stop=sp_)
                nc.tensor.matmul(zm[0:rows, PSG:PSG + 361], lhsT=lhs,
                                 rhs=wt_tiles[kc][:, GP:GP + 361], start=st_, stop=sp_)
                nc.tensor.matmul(zl[0:rows, 0:361], lhsT=lhs,
                                 rhs=wt_tiles[kc][:, 2 * GP:2 * GP + 361],
                                 start=st_, stop=sp_)
            # loop-group epilogue pieces (early, straight from PSUM)
            sgl = tpool.tile([128, 2], F32, tag="sgl", name=f"sgl{s}_{mt}")
            nc.scalar.activation(sgl[0:rows, 0:1], zl[0:rows, 360:361], AF.Sigmoid)
            t1 = tpool.tile([128, DOUT], BF16, tag="t1", name=f"t1{s}_{mt}")
            nc.vector.tensor_scalar_mul(t1[0:rows], zl[0:rows, 0:360], sgl[0:rows, 0:1])
            # main Z copy: in/out groups -> zs block
            zs = st["zs"]
            src = zm[0:rows, :].rearrange("p (g c) -> p g c", g=2)[:, :, 0:GP]
            dst = zs[0:rows, mt * B2:(mt + 1) * B2].rearrange("p (g c) -> p g c", g=2)
            nc.scalar.copy(dst, src)
            st[("t1", mt)] = t1

        def emit_agg(s, mt):
            st = state[s]
            mfb_t, zs, ot = st["mfb"], st["zs"], st["ot"]
            t1 = st.pop(("t1", mt))
            rows = 128 if mt == 0 else 72
            d0 = mt * 128
            ap_ = apool.tile([128, 2 * PSG], F32, tag="ap", name=f"ap{s}_{mt}")
            nc.tensor.matmul(ap_[0:rows, 0:361], lhsT=mfb_t[0:128, d0:d0 + rows],
                             rhs=zs[0:128, 0:361], start=True, stop=False)
            nc.tensor.matmul(ap_[0:rows, 0:361], lhsT=mfb_t[0:72, 200 + d0:200 + d0 + rows],
                             rhs=zs[0:72, B2:B2 + 361], start=False, stop=True)
            nc.tensor.matmul(ap_[0:rows, PSG:PSG + 361], lhsT=mfb_t[0:128, 400 + d0:400 + d0 + rows],
                             rhs=zs[0:128, GP:GP + 361], start=True, stop=False)
            nc.tensor.matmul(ap_[0:rows, PSG:PSG + 361], lhsT=mfb_t[0:72, 600 + d0:600 + d0 + rows],
                             rhs=zs[0:72, B2 + GP:B2 + GP + 361], start=False, stop=True)

            sgt = tpool.tile([128, 2], F32, tag="sgt", name=f"sg{s}_{mt}")
            nc.scalar.activation(
                sgt[0:rows, 0:2],
                ap_[0:rows, :].rearrange("p (g c) -> p g c", g=2)[:, :, 360],
                AF.Sigmoid)
            if s == SPC - 1 and mt == 1:
                st["tail"] = (ap_, sgt, t1)
                return
            t2 = tpool.tile([128, DOUT], BF16, tag="t2", name=f"t2{s}_{mt}")
            t3 = tpool.tile([128, DOUT], BF16, tag="t3", name=f"t3{s}_{mt}")
            nc.vector.scalar_tensor_tensor(
                out=t2[0:rows], in0=ap_[0:rows, 0:360],
                scalar=sgt[0:rows, 0:1], in1=t1[0:rows],
                op0=OP.mult, op1=OP.add)
            nc.vector.scalar_tensor_tensor(
                out=t3[0:rows], in0=ap_[0:rows, PSG:PSG + 360],
                scalar=sgt[0:rows, 1:2], in1=t2[0:rows],
                op0=OP.mult, op1=OP.add)
            st[("t3", mt)] = t3

        def emit_relu(s, mt):
            # deferred one pipeline step: keeps Z copies (PE-critical) at the
            # head of scalar's in-order queue, relu/DMA are latency-tolerant
            st = state[s]
            rows = 128 if mt == 0 else 72
            ot = st["ot"]
            t3 = st.pop(("t3", mt))
            nc.scalar.activation(
                ot[0:rows, mt * DOUT:(mt + 1) * DOUT], t3[0:rows], AF.Relu)
            if s == SPC - 1:
                nc.sync.dma_start(
                    out=out_d[s, 0:rows, mt * DOUT:(mt + 1) * DOUT],
                    in_=ot[0:rows, mt * DOUT:(mt + 1) * DOUT])
                if mt == 1:
                    state.pop(s)
            elif mt == 1:
                nc.gpsimd.dma_start(out=out_d[s], in_=ot)
                state.pop(s)

        def emit_tail(s):
            # last unit: halved epilogue chain pipelines STT/relu/DMA
            st = state[s]
            ap_, sgt, t1 = st.pop("tail")
            ot = st["ot"]
            rows, H = 72, 180
            t2 = tpool.tile([128, DOUT], BF16, tag="t2", name=f"t2{s}_tl")
            t3 = tpool.tile([128, DOUT], BF16, tag="t3", name=f"t3{s}_tl")
            for h in range(2):
                cs = slice(h * H, (h + 1) * H)
                nc.vector.scalar_tensor_tensor(
                    out=t2[0:rows, cs], in0=ap_[0:rows, h * H:h * H + H],
                    scalar=sgt[0:rows, 0:1], in1=t1[0:rows, cs],
                    op0=OP.mult, op1=OP.add)
                nc.vector.scalar_tensor_tensor(
                    out=t3[0:rows, cs], in0=ap_[0:rows, PSG + h * H:PSG + h * H + H],
                    scalar=sgt[0:rows, 1:2], in1=t2[0:rows, cs],
                    op0=OP.mult, op1=OP.add)
                nc.scalar.activation(
                    ot[0:rows, DOUT + h * H:DOUT + (h + 1) * H],
                    t3[0:rows, cs], AF.Relu)
                nc.sync.dma_start(
                    out=out_d[s, 0:rows, DOUT + h * H:DOUT + (h + 1) * H],
                    in_=ot[0:rows, DOUT + h * H:DOUT + (h + 1) * H])
            state.pop(s)

        # ---- software pipeline: aggregation lags main by one sentence,
        # relu/output-DMA lag one more step ----
        for s in range(SPC):
            emit_main(s, 0)
            if s > 0:
                emit_agg(s - 1, 0)
            if s > 1:
                emit_relu(s - 2, 1)
            emit_main(s, 1)
            if s == SPC - 1:
                emit_agg(s, 0)          # shorten the drain: last sentence's
                emit_agg(s - 1, 1)      # mt0 slots in before the stragglers
                emit_relu(s - 1, 0)
                emit_agg(s, 1)
                emit_relu(s - 1, 1)
                emit_relu(s, 0)
                emit_tail(s)
            elif s > 0:
                emit_agg(s - 1, 1)
                emit_relu(s - 1, 0)

    nc.compile()
    return nc


def _get_compiled():
    global _compiled
    if _compiled is None:
        _compiled = _build()
    return _compiled


def kernel(gcn_in, adj_ind, adj_data, w_in, b_in, w_out, b_out, w_loop,
           w_gin, b_gin, w_gout, b_gout, w_gloop):
    from concourse.bass_utils import run_bass_kernel_spmd

    x = np.asarray(gcn_in, np.float32)           # [B, N, DIN]
    idx = np.asarray(adj_ind)[0]                 # [B, E, 2] int
    dat = np.asarray(adj_data, np.float32)[0]    # [B, E]

    # fused weight matrix with bias row
    wt = np.zeros((KA, 3 * GP), np.float32)
    for g, (w, gw, bias, gb) in enumerate([
        (w_in, w_gin, b_in, b_gin),
        (w_out, w_gout, b_out, b_gout),
        (w_loop, w_gloop, None, None),
    ]):
        wt[0:DIN, g * GP:g * GP + DOUT] = np.asarray(w, np.float32)
        wt[0:DIN, g * GP + DOUT] = np.asarray(gw, np.float32)[:, 0]
        if bias is not None:
            wt[DIN, g * GP:g * GP + DOUT] = np.asarray(bias, np.float32)[0]
            wt[DIN, g * GP + DOUT] = np.asarray(gb, np.float32)[0]
    wt = wt.astype(NBF16)

    # dense per-sentence adjacency matrices
    M = np.zeros((B, NN, NN), np.float32)
    bi = np.broadcast_to(np.arange(B)[:, None], idx.shape[:2])
    np.add.at(M, (bi, idx[:, :, 0].astype(np.int64), idx[:, :, 1].astype(np.int64)), dat)

    def chunked(mm):      # [SPC,200,200] -> [SPC,128,400]: two 128-row src chunks
        out = np.zeros((SPC, 128, 2 * NN), np.float32)
        out[:, :, 0:NN] = mm[:, 0:128, :]
        out[:, 0:72, NN:2 * NN] = mm[:, 128:200, :]
        return out

    nc = _get_compiled()
    in_maps = []
    for c in range(NCORES):
        xc = x[c * SPC:(c + 1) * SPC].reshape(ROWS, DIN)
        xt = np.empty((KA, ROWS), np.float32)
        xt[0:DIN] = xc.T
        xt[DIN] = 1.0
        mc = M[c * SPC:(c + 1) * SPC]
        mfb = np.concatenate(
            [chunked(mc), chunked(np.ascontiguousarray(mc.transpose(0, 2, 1)))],
            axis=2)
        in_maps.append({
            "xt": np.ascontiguousarray(xt).astype(NBF16),
            "wt": wt,
            "mfb": np.ascontiguousarray(mfb).astype(NBF16),
        })

    res = run_bass_kernel_spmd(nc, in_maps, core_ids=list(range(NCORES)))
    kernel.last_results = res
    out = np.empty((B, NN, DOUT), np.float32)
    for c in range(NCORES):
        oc = res.results[c]["out"].astype(np.float32)   # [SPC,128,720]
        oc_s = out[c * SPC:(c + 1) * SPC]               # [SPC,200,360]
        oc_s[:, 0:128, :] = oc[:, :, 0:DOUT]
        oc_s[:, 128:200, :] = oc[:, 0:72, DOUT:2 * DOUT]
    return out


# revision 8
# speedup vs baseline: 1.0227x; 1.0162x over previous
"""Gated GCN layer (DDI message passing) on 8 Trainium2 NeuronCores — v3.

Data-parallel over batch B=256 -> 32 sentences/core. Host builds dense
per-sentence adjacency M (and M^T); aggregations are dense matmuls.

v3 vs baseline:
  - Aggregation results stay in PSUM (ap, bufs=2); the gating epilogue
    reads them directly — the baseline's agg PSUM->SBUF copy is gone.
  - The loop group of the main GEMM also stays in PSUM (zloop, bufs=2):
    its gate sigmoid and t1 = loopZ*sigma_loop read PSUM directly, so
    the main Z copy shrinks to the in/out groups (728 cols vs 1092).
  - zmain (in/out groups) single-buffered: its copy drains during the
    next agg+main bursts. PSUM: 2 + 2 + 4 = 8 banks.
  - Epilogue ops only on scalar (sigmoids, relu) and vector (t1 mul,
    two fused scalar_tensor_tensor) — gpsimd elementwise is ~60x
    slower than DVE and is used for nothing; sync/gpsimd drive DMA.

SBUF zs layout per sentence: [128, 2 blocks x 2 groups x 364] bf16,
block0 = node rows 0:128, block1 = rows 128:200 (partitions 0:72).
Group: 360 feats + gate col at 360. Groups: in | out.
"""
import sys

if "/opt/trn_rl_repo" not in sys.path:
    sys.path.insert(0, "/opt/trn_rl_repo")

from contextlib import ExitStack

import ml_dtypes
import numpy as np

B, NN, EE, DIN, DOUT = 256, 200, 400, 360, 360
NCORES = 8
SPC = B // NCORES          # 32 sentences per core
ROWS = SPC * NN            # 6400 rows per core
KA = DIN + 1               # 361: augmented contraction (ones row for bias)
GP = 364                   # group stride in SBUF (4B aligned for bf16)
B2 = 2 * GP                # block stride in zs (in/out groups only)
PSG = 512                  # PSUM bank stride (fp32)
KCH = [(0, 121), (121, 241), (241, 361)]   # K chunks <= 128
NBF16 = np.dtype(ml_dtypes.bfloat16)

_compiled = None


def _build():
    import concourse.bacc as bacc
    import concourse.mybir as mybir
    from concourse.tile import TileContext

    BF16 = mybir.dt.bfloat16
    F32 = mybir.dt.float32
    AF = mybir.ActivationFunctionType
    OP = mybir.AluOpType

    WOF = 3 * GP             # wt columns prepended to each xt row
    nc = bacc.Bacc(None, target_bir_lowering=False)
    xt_d = nc.dram_tensor("xt", [KA, WOF + ROWS], BF16, kind="ExternalInput")
    mfb_d = nc.dram_tensor("mfb", [SPC, 128, 800], BF16, kind="ExternalInput")
    out_d = nc.dram_tensor("out", [SPC, 128, 2 * DOUT], BF16, kind="ExternalOutput")

    with TileContext(nc) as tc, ExitStack() as ctx:
        cpool = ctx.enter_context(tc.tile_pool(name="cpool", bufs=1))
        mpool = ctx.enter_context(tc.tile_pool(name="mpool", bufs=32))
        spool = ctx.enter_context(tc.tile_pool(name="spool", bufs=5))
        tpool = ctx.enter_context(tc.tile_pool(name="tpool", bufs=5))
        zmpool = ctx.enter_context(tc.tile_pool(name="zmpool", bufs=1, space="PSUM"))
        zlpool = ctx.enter_context(tc.tile_pool(name="zlpool", bufs=2, space="PSUM"))
        apool = ctx.enter_context(tc.tile_pool(name="apool", bufs=2, space="PSUM"))

        # ---- resident inputs: smallest pieces first so matmul 1 can
        # start ASAP; each kc chunk rides its own engine queue ----
        eng3 = [nc.scalar, nc.gpsimd, nc.sync]
        xtw_tiles = [cpool.tile([k1 - k0, WOF + ROWS], BF16, name=f"xtw{kc}")
                     for kc, (k0, k1) in enumerate(KCH)]
        wt_tiles = [t[:, 0:WOF] for t in xtw_tiles]
        xt_tiles = [t[:, WOF:WOF + ROWS] for t in xtw_tiles]

        state = {}       # sentence -> dict(mfb, zs, ot, t1 per mt)

        def get_state(s):
            if s not in state:
                mfb_t = mpool.tile([128, 800], BF16, tag="mfb", name=f"mfb{s}")
                nc.sync.dma_start(out=mfb_t, in_=mfb_d[s])
                state[s] = {"mfb": mfb_t}
            return state[s]

        # warmup buffer first: memset rides ahead of gpsimd's DMA queue
        wup = cpool.tile([128, 512], BF16, name="wup")
        nc.gpsimd.memset(wup, 0.0)

        for kc, (k0, k1) in enumerate(KCH):
            eng3[kc].dma_start(out=xtw_tiles[kc][:, 0:WOF + 400],
                               in_=xt_d[k0:k1, 0:WOF + 400])
        for s in range(4):
            get_state(s)

        # rest of xt all on gpsimd: it is otherwise idle until the first
        # output DMAs (~20us in), and keeping sync free keeps mfb flowing
        for a, b in [(400, 1200), (1200, 2400), (2400, 3800),
                     (3800, 5100), (5100, ROWS)]:
            for kc, (k0, k1) in enumerate(KCH):
                nc.gpsimd.dma_start(out=xt_tiles[kc][:, a:b],
                                    in_=xt_d[k0:k1, WOF + a:WOF + b])
        # second prefetch wave after the boot-critical transfers
        for s in range(4, SPC):
            get_state(s)

        # ---- PE warmup: hold the activity monitor at full clock while
        # the first input DMAs land (junk matmuls on zeroed SBUF) ----
        wps = apool.tile([128, 2 * PSG], F32, tag="ap", name="wps")
        for i in range(15):
            nc.tensor.matmul(wps[:, 0:512], lhsT=wup[:, 0:128],
                             rhs=wup[:, 0:512], start=True, stop=True)

        def emit_main(s, mt):
            rows = 128 if mt == 0 else 72
            c0 = s * NN + mt * 128
            st = get_state(s)
            if "zs" not in st:
                st["zs"] = spool.tile([128, 2 * B2], BF16, tag="zs", name=f"zs{s}")
                st["ot"] = tpool.tile([128, 2 * DOUT], BF16, tag="ot", name=f"ot{s}")
            zm = zmpool.tile([128, 2 * PSG], F32, tag="zm", name=f"zm{s}_{mt}")
            zl = zlpool.tile([128, PSG], F32, tag="zl", name=f"zl{s}_{mt}")
            for kc, (k0, k1) in enumerate(KCH):
                lhs = xt_tiles[kc][:, c0:c0 + rows]
                st_, sp_ = (kc == 0), (kc == 2)
                nc.tensor.matmul(zm[0:rows, 0:361], lhsT=lhs,
                                 rhs=wt_tiles[kc][:, 0:361], start=st_, stop=sp_)
                nc.tensor.matmul(zm[0:rows, PSG:PSG + 361], lhsT=lhs,
                                 rhs=wt_tiles[kc][:, GP:GP + 361], start=st_, stop=sp_)
                nc.tensor.matmul(zl[0:rows, 0:361], lhsT=lhs,
                                 rhs=wt_tiles[kc][:, 2 * GP:2 * GP + 361],
                                 start=st_, stop=sp_)
            # loop-group epilogue pieces (early, straight from PSUM)
            sgl = tpool.tile([128, 2], F32, tag="sgl", name=f"sgl{s}_{mt}")
            nc.scalar.activation(sgl[0:rows, 0:1], zl[0:rows, 360:361], AF.Sigmoid)
            t1 = tpool.tile([128, DOUT], BF16, tag="t1", name=f"t1{s}_{mt}")
            nc.vector.tensor_scalar_mul(t1[0:rows], zl[0:rows, 0:360], sgl[0:rows, 0:1])
            # main Z copy: in/out groups -> zs block
            zs = st["zs"]
            src = zm[0:rows, :].rearrange("p (g c) -> p g c", g=2)[:, :, 0:GP]
            dst = zs[0:rows, mt * B2:(mt + 1) * B2].rearrange("p (g c) -> p g c", g=2)
            nc.scalar.copy(dst, src)
            st[("t1", mt)] = t1

        def emit_agg(s, mt):
            st = state[s]
            mfb_t, zs, ot = st["mfb"], st["zs"], st["ot"]
            t1 = st.pop(("t1", mt))
            rows = 128 if mt == 0 else 72
            d0 = mt * 128
            ap_ = apool.tile([128, 2 * PSG], F32, tag="ap", name=f"ap{s}_{mt}")
            nc.tensor.matmul(ap_[0:rows, 0:361], lhsT=mfb_t[0:128, d0:d0 + rows],
                             rhs=zs[0:128, 0:361], start=True, stop=False)
            nc.tensor.matmul(ap_[0:rows, 0:361], lhsT=mfb_t[0:72, 200 + d0:200 + d0 + rows],
                             rhs=zs[0:72, B2:B2 + 361], start=False, stop=True)
            nc.tensor.matmul(ap_[0:rows, PSG:PSG + 361], lhsT=mfb_t[0:128, 400 + d0:400 + d0 + rows],
                             rhs=zs[0:128, GP:GP + 361], start=True, stop=False)
            nc.tensor.matmul(ap_[0:rows, PSG:PSG + 361], lhsT=mfb_t[0:72, 600 + d0:600 + d0 + rows],
                             rhs=zs[0:72, B2 + GP:B2 + GP + 361], start=False, stop=True)

            sgt = tpool.tile([128, 2], F32, tag="sgt", name=f"sg{s}_{mt}")
            nc.scalar.activation(
                sgt[0:rows, 0:2],
                ap_[0:rows, :].rearrange("p (g c) -> p g c", g=2)[:, :, 360],
                AF.Sigmoid)
            t2 = tpool.tile([128, DOUT], BF16, tag="t2", name=f"t2{s}_{mt}")
            t3 = tpool.tile([128, DOUT], BF16, tag="t3", name=f"t3{s}_{mt}")
            nc.vector.scalar_tensor_tensor(
                out=t2[0:rows], in0=ap_[0:rows, 0:360],
                scalar=sgt[0:rows, 0:1], in1=t1[0:rows],
                op0=OP.mult, op1=OP.add)
            nc.vector.scalar_tensor_tensor(
                out=t3[0:rows], in0=ap_[0:rows, PSG:PSG + 360],
                scalar=sgt[0:rows, 1:2], in1=t2[0:rows],
                op0=OP.mult, op1=OP.add)
            st[("t3", mt)] = t3

        def emit_relu(s, mt):
            # deferred one pipeline step: keeps Z copies (PE-critical) at the
            # head of scalar's in-order queue, relu/DMA are latency-tolerant
            st = state[s]
            rows = 128 if mt == 0 else 72
            ot = st["ot"]
            t3 = st.pop(("t3", mt))
            nc.scalar.activation(
                ot[0:rows, mt * DOUT:(mt + 1) * DOUT], t3[0:rows], AF.Relu)
            if s >= SPC - 2:
                # last two sentences drain per row-block on the fast ring
                nc.sync.dma_start(
                    out=out_d[s, 0:rows, mt * DOUT:(mt + 1) * DOUT],
                    in_=ot[0:rows, mt * DOUT:(mt + 1) * DOUT])
                if mt == 1:
                    state.pop(s)
            elif mt == 1:
                nc.gpsimd.dma_start(out=out_d[s], in_=ot)
                state.pop(s)

        def emit_tail(s):
            # last unit fully fused: STT1 runs behind the out-direction
            # matmuls, then halved STT2/relu/DMA pipeline the drain.
            # zm is free after the last main tile's copy.
            st = state[s]
            mfb_t, zs, ot = st["mfb"], st["zs"], st["ot"]
            t1 = st.pop(("t1", 1))
            rows, H, d0 = 72, 180, 128
            ap_ = zmpool.tile([128, 2 * PSG], F32, tag="zm", name=f"ap{s}_tl")
            nc.tensor.matmul(ap_[0:rows, 0:361], lhsT=mfb_t[0:128, d0:d0 + rows],
                             rhs=zs[0:128, 0:361], start=True, stop=False)
            nc.tensor.matmul(ap_[0:rows, 0:361], lhsT=mfb_t[0:72, 200 + d0:200 + d0 + rows],
                             rhs=zs[0:72, B2:B2 + 361], start=False, stop=True)
            sgt = tpool.tile([128, 2], F32, tag="sgt", name=f"sg{s}_tl")
            nc.scalar.activation(sgt[0:rows, 0:1], ap_[0:rows, 360:361], AF.Sigmoid)
            t2 = tpool.tile([128, DOUT], BF16, tag="t2", name=f"t2{s}_tl")
            t3 = tpool.tile([128, DOUT], BF16, tag="t3", name=f"t3{s}_tl")
            for h in range(2):
                cs = slice(h * H, (h + 1) * H)
                nc.vector.scalar_tensor_tensor(
                    out=t2[0:rows, cs], in0=ap_[0:rows, h * H:h * H + H],
                    scalar=sgt[0:rows, 0:1], in1=t1[0:rows, cs],
                    op0=OP.mult, op1=OP.add)
            nc.tensor.matmul(ap_[0:rows, PSG:PSG + 361], lhsT=mfb_t[0:128, 400 + d0:400 + d0 + rows],
                             rhs=zs[0:128, GP:GP + 361], start=True, stop=False)
            nc.tensor.matmul(ap_[0:rows, PSG:PSG + 361], lhsT=mfb_t[0:72, 600 + d0:600 + d0 + rows],
                             rhs=zs[0:72, B2 + GP:B2 + GP + 361], start=False, stop=True)
            nc.scalar.activation(sgt[0:rows, 1:2], ap_[0:rows, PSG + 360:PSG + 361],
                                 AF.Sigmoid)
            for h in range(2):
                cs = slice(h * H, (h + 1) * H)
                nc.vector.scalar_tensor_tensor(
                    out=t3[0:rows, cs], in0=ap_[0:rows, PSG + h * H:PSG + h * H + H],
                    scalar=sgt[0:rows, 1:2], in1=t2[0:rows, cs],
                    op0=OP.mult, op1=OP.add)
                nc.scalar.activation(
                    ot[0:rows, DOUT + h * H:DOUT + (h + 1) * H],
                    t3[0:rows, cs], AF.Relu)
                nc.sync.dma_start(
                    out=out_d[s, 0:rows, DOUT + h * H:DOUT + (h + 1) * H],
                    in_=ot[0:rows, DOUT + h * H:DOUT + (h + 1) * H])
            state.pop(s)

        # ---- software pipeline: aggregation lags main by one sentence,
        # relu/output-DMA lag one more step ----
        for s in range(SPC):
            emit_main(s, 0)
            if s > 0:
                emit_agg(s - 1, 0)
            if s > 1:
                emit_relu(s - 2, 1)
            emit_main(s, 1)
            if s == SPC - 1:
                emit_agg(s - 1, 1)      # sentence 30 drains first: its
                emit_relu(s - 1, 0)     # output DMA is the long pole
                emit_agg(s, 0)
                emit_relu(s - 1, 1)
                emit_relu(s, 0)
                emit_tail(s)
            elif s > 0:
                emit_agg(s - 1, 1)
                emit_relu(s - 1, 0)

    nc.compile()
    return nc


def _get_compiled():
    global _compiled
    if _compiled is None:
        _compiled = _build()
    return _compiled


def kernel(gcn_in, adj_ind, adj_data, w_in, b_in, w_out, b_out, w_loop,
           w_gin, b_gin, w_gout, b_gout, w_gloop):
    from concourse.bass_utils import run_bass_kernel_spmd

    x = np.asarray(gcn_in, np.float32)           # [B, N, DIN]
    idx = np.asarray(adj_ind)[0]                 # [B, E, 2] int
    dat = np.asarray(adj_data, np.float32)[0]    # [B, E]

    # fused weight matrix with bias row
    wt = np.zeros((KA, 3 * GP), np.float32)
    for g, (w, gw, bias, gb) in enumerate([
        (w_in, w_gin, b_in, b_gin),
        (w_out, w_gout, b_out, b_gout),
        (w_loop, w_gloop, None, None),
    ]):
        wt[0:DIN, g * GP:g * GP + DOUT] = np.asarray(w, np.float32)
        wt[0:DIN, g * GP + DOUT] = np.asarray(gw, np.float32)[:, 0]
        if bias is not None:
            wt[DIN, g * GP:g * GP + DOUT] = np.asarray(bias, np.float32)[0]
            wt[DIN, g * GP + DOUT] = np.asarray(gb, np.float32)[0]
    wt = wt.astype(NBF16)

    # dense per-sentence adjacency matrices
    M = np.zeros((B, NN, NN), np.float32)
    bi = np.broadcast_to(np.arange(B)[:, None], idx.shape[:2])
    np.add.at(M, (bi, idx[:, :, 0].astype(np.int64), idx[:, :, 1].astype(np.int64)), dat)

    def chunked(mm):      # [SPC,200,200] -> [SPC,128,400]: two 128-row src chunks
        out = np.zeros((SPC, 128, 2 * NN), np.float32)
        out[:, :, 0:NN] = mm[:, 0:128, :]
        out[:, 0:72, NN:2 * NN] = mm[:, 128:200, :]
        return out

    nc = _get_compiled()
    in_maps = []
    for c in range(NCORES):
        xc = x[c * SPC:(c + 1) * SPC].reshape(ROWS, DIN)
        xt = np.empty((KA, ROWS), np.float32)
        xt[0:DIN] = xc.T
        xt[DIN] = 1.0
        mc = M[c * SPC:(c + 1) * SPC]
        mfb = np.concatenate(
            [chunked(mc), chunked(np.ascontiguousarray(mc.transpose(0, 2, 1)))],
            axis=2)
        xtw = np.concatenate([wt.astype(np.float32), xt], axis=1)
        in_maps.append({
            "xt": np.ascontiguousarray(xtw).astype(NBF16),
            "mfb": np.ascontiguousarray(mfb).astype(NBF16),
        })

    res = run_bass_kernel_spmd(nc, in_maps, core_ids=list(range(NCORES)))
    kernel.last_results = res
    out = np.empty((B, NN, DOUT), np.float32)
    for c in range(NCORES):
        oc = res.results[c]["out"].astype(np.float32)   # [SPC,128,720]
        oc_s = out[c * SPC:(c + 1) * SPC]               # [SPC,200,360]
        oc_s[:, 0:128, :] = oc[:, :, 0:DOUT]
        oc_s[:, 128:200, :] = oc[:, 0:72, DOUT:2 * DOUT]
    return out


# revision 9
# speedup vs baseline: 1.0251x; 1.0023x over previous
"""Gated GCN layer (DDI message passing) on 8 Trainium2 NeuronCores — v3.

Data-parallel over batch B=256 -> 32 sentences/core. Host builds dense
per-sentence adjacency M (and M^T); aggregations are dense matmuls.

v3 vs baseline:
  - Aggregation results stay in PSUM (ap, bufs=2); the gating epilogue
    reads them directly — the baseline's agg PSUM->SBUF copy is gone.
  - The loop group of the main GEMM also stays in PSUM (zloop, bufs=2):
    its gate sigmoid and t1 = loopZ*sigma_loop read PSUM directly, so
    the main Z copy shrinks to the in/out groups (728 cols vs 1092).
  - zmain (in/out groups) single-buffered: its copy drains during the
    next agg+main bursts. PSUM: 2 + 2 + 4 = 8 banks.
  - Epilogue ops only on scalar (sigmoids, relu) and vector (t1 mul,
    two fused scalar_tensor_tensor) — gpsimd elementwise is ~60x
    slower than DVE and is used for nothing; sync/gpsimd drive DMA.

SBUF zs layout per sentence: [128, 2 blocks x 2 groups x 364] bf16,
block0 = node rows 0:128, block1 = rows 128:200 (partitions 0:72).
Group: 360 feats + gate col at 360. Groups: in | out.
"""
import sys

if "/opt/trn_rl_repo" not in sys.path:
    sys.path.insert(0, "/opt/trn_rl_repo")

from contextlib import ExitStack

import ml_dtypes
import numpy as np

B, NN, EE, DIN, DOUT = 256, 200, 400, 360, 360
NCORES = 8
SPC = B // NCORES          # 32 sentences per core
ROWS = SPC * NN            # 6400 rows per core
KA = DIN + 1               # 361: augmented contraction (ones row for bias)
GP = 364                   # group stride in SBUF (4B aligned for bf16)
B2 = 2 * GP                # block stride in zs (in/out groups only)
PSG = 512                  # PSUM bank stride (fp32)
KCH = [(0, 121), (121, 241), (241, 361)]   # K chunks <= 128
NBF16 = np.dtype(ml_dtypes.bfloat16)

_compiled = None


def _build():
    import concourse.bacc as bacc
    import concourse.mybir as mybir
    from concourse.tile import TileContext

    BF16 = mybir.dt.bfloat16
    F32 = mybir.dt.float32
    AF = mybir.ActivationFunctionType
    OP = mybir.AluOpType

    WOF = 3 * GP             # wt columns prepended to each xt row
    nc = bacc.Bacc(None, target_bir_lowering=False)
    xt_d = nc.dram_tensor("xt", [KA, WOF + ROWS], BF16, kind="ExternalInput")
    mfb_d = nc.dram_tensor("mfb", [SPC, 128, 800], BF16, kind="ExternalInput")
    out_d = nc.dram_tensor("out", [SPC, 128, 2 * DOUT], BF16, kind="ExternalOutput")

    with TileContext(nc) as tc, ExitStack() as ctx:
        cpool = ctx.enter_context(tc.tile_pool(name="cpool", bufs=1))
        mpool = ctx.enter_context(tc.tile_pool(name="mpool", bufs=32))
        spool = ctx.enter_context(tc.tile_pool(name="spool", bufs=5))
        tpool = ctx.enter_context(tc.tile_pool(name="tpool", bufs=5))
        zmpool = ctx.enter_context(tc.tile_pool(name="zmpool", bufs=1, space="PSUM"))
        zlpool = ctx.enter_context(tc.tile_pool(name="zlpool", bufs=2, space="PSUM"))
        apool = ctx.enter_context(tc.tile_pool(name="apool", bufs=2, space="PSUM"))

        # ---- resident inputs: smallest pieces first so matmul 1 can
        # start ASAP; each kc chunk rides its own engine queue ----
        eng3 = [nc.scalar, nc.gpsimd, nc.sync]
        xtw_tiles = [cpool.tile([k1 - k0, WOF + ROWS], BF16, name=f"xtw{kc}")
                     for kc, (k0, k1) in enumerate(KCH)]
        wt_tiles = [t[:, 0:WOF] for t in xtw_tiles]
        xt_tiles = [t[:, WOF:WOF + ROWS] for t in xtw_tiles]

        state = {}       # sentence -> dict(mfb, zs, ot, t1 per mt)

        def get_state(s):
            if s not in state:
                mfb_t = mpool.tile([128, 800], BF16, tag="mfb", name=f"mfb{s}")
                nc.sync.dma_start(out=mfb_t, in_=mfb_d[s])
                state[s] = {"mfb": mfb_t}
            return state[s]

        # warmup buffer first: memset rides ahead of gpsimd's DMA queue
        wup = cpool.tile([128, 512], BF16, name="wup")
        nc.gpsimd.memset(wup, 0.0)

        for kc, (k0, k1) in enumerate(KCH):
            eng3[kc].dma_start(out=xtw_tiles[kc][:, 0:WOF + 400],
                               in_=xt_d[k0:k1, 0:WOF + 400])
        for s in range(4):
            get_state(s)

        # rest of xt all on gpsimd: it is otherwise idle until the first
        # output DMAs (~20us in), and keeping sync free keeps mfb flowing
        for a, b in [(400, 1200), (1200, 2400), (2400, 3800),
                     (3800, 5100), (5100, ROWS)]:
            for kc, (k0, k1) in enumerate(KCH):
                nc.gpsimd.dma_start(out=xt_tiles[kc][:, a:b],
                                    in_=xt_d[k0:k1, WOF + a:WOF + b])
        # second prefetch wave after the boot-critical transfers
        for s in range(4, SPC):
            get_state(s)

        # ---- PE warmup: hold the activity monitor at full clock while
        # the first input DMAs land (junk matmuls on zeroed SBUF) ----
        wps = apool.tile([128, 2 * PSG], F32, tag="ap", name="wps")
        for i in range(15):
            nc.tensor.matmul(wps[:, 0:512], lhsT=wup[:, 0:128],
                             rhs=wup[:, 0:512], start=True, stop=True)

        def emit_main(s, mt):
            rows = 128 if mt == 0 else 72
            c0 = s * NN + mt * 128
            st = get_state(s)
            if "zs" not in st:
                st["zs"] = spool.tile([128, 2 * B2], BF16, tag="zs", name=f"zs{s}")
                st["ot"] = tpool.tile([128, 2 * DOUT], BF16, tag="ot", name=f"ot{s}")
            zm = zmpool.tile([128, 2 * PSG], F32, tag="zm", name=f"zm{s}_{mt}")
            zl = zlpool.tile([128, PSG], F32, tag="zl", name=f"zl{s}_{mt}")
            for kc, (k0, k1) in enumerate(KCH):
                lhs = xt_tiles[kc][:, c0:c0 + rows]
                st_, sp_ = (kc == 0), (kc == 2)
                # loop group first: its sigmoid+t1 chain is the long pole
                nc.tensor.matmul(zl[0:rows, 0:361], lhsT=lhs,
                                 rhs=wt_tiles[kc][:, 2 * GP:2 * GP + 361],
                                 start=st_, stop=sp_)
                nc.tensor.matmul(zm[0:rows, 0:361], lhsT=lhs,
                                 rhs=wt_tiles[kc][:, 0:361], start=st_, stop=sp_)
                nc.tensor.matmul(zm[0:rows, PSG:PSG + 361], lhsT=lhs,
                                 rhs=wt_tiles[kc][:, GP:GP + 361], start=st_, stop=sp_)
            # loop-group epilogue pieces (early, straight from PSUM)
            sgl = tpool.tile([128, 2], F32, tag="sgl", name=f"sgl{s}_{mt}")
            nc.scalar.activation(sgl[0:rows, 0:1], zl[0:rows, 360:361], AF.Sigmoid)
            t1 = tpool.tile([128, DOUT], BF16, tag="t1", name=f"t1{s}_{mt}")
            nc.vector.tensor_scalar_mul(t1[0:rows], zl[0:rows, 0:360], sgl[0:rows, 0:1])
            # main Z copy: in/out groups -> zs block
            zs = st["zs"]
            src = zm[0:rows, :].rearrange("p (g c) -> p g c", g=2)[:, :, 0:GP]
            dst = zs[0:rows, mt * B2:(mt + 1) * B2].rearrange("p (g c) -> p g c", g=2)
            nc.scalar.copy(dst, src)
            st[("t1", mt)] = t1

        def emit_agg(s, mt):
            st = state[s]
            mfb_t, zs, ot = st["mfb"], st["zs"], st["ot"]
            t1 = st.pop(("t1", mt))
            rows = 128 if mt == 0 else 72
            d0 = mt * 128
            ap_ = apool.tile([128, 2 * PSG], F32, tag="ap", name=f"ap{s}_{mt}")
            nc.tensor.matmul(ap_[0:rows, 0:361], lhsT=mfb_t[0:128, d0:d0 + rows],
                             rhs=zs[0:128, 0:361], start=True, stop=False)
            nc.tensor.matmul(ap_[0:rows, 0:361], lhsT=mfb_t[0:72, 200 + d0:200 + d0 + rows],
                             rhs=zs[0:72, B2:B2 + 361], start=False, stop=True)
            nc.tensor.matmul(ap_[0:rows, PSG:PSG + 361], lhsT=mfb_t[0:128, 400 + d0:400 + d0 + rows],
                             rhs=zs[0:128, GP:GP + 361], start=True, stop=False)
            nc.tensor.matmul(ap_[0:rows, PSG:PSG + 361], lhsT=mfb_t[0:72, 600 + d0:600 + d0 + rows],
                             rhs=zs[0:72, B2 + GP:B2 + GP + 361], start=False, stop=True)

            sgt = tpool.tile([128, 2], F32, tag="sgt", name=f"sg{s}_{mt}")
            nc.scalar.activation(
                sgt[0:rows, 0:2],
                ap_[0:rows, :].rearrange("p (g c) -> p g c", g=2)[:, :, 360],
                AF.Sigmoid)
            t2 = tpool.tile([128, DOUT], BF16, tag="t2", name=f"t2{s}_{mt}")
            t3 = tpool.tile([128, DOUT], BF16, tag="t3", name=f"t3{s}_{mt}")
            nc.vector.scalar_tensor_tensor(
                out=t2[0:rows], in0=ap_[0:rows, 0:360],
                scalar=sgt[0:rows, 0:1], in1=t1[0:rows],
                op0=OP.mult, op1=OP.add)
            nc.vector.scalar_tensor_tensor(
                out=t3[0:rows], in0=ap_[0:rows, PSG:PSG + 360],
                scalar=sgt[0:rows, 1:2], in1=t2[0:rows],
                op0=OP.mult, op1=OP.add)
            st[("t3", mt)] = t3

        def emit_relu(s, mt):
            # deferred one pipeline step: keeps Z copies (PE-critical) at the
            # head of scalar's in-order queue, relu/DMA are latency-tolerant
            st = state[s]
            rows = 128 if mt == 0 else 72
            ot = st["ot"]
            t3 = st.pop(("t3", mt))
            nc.scalar.activation(
                ot[0:rows, mt * DOUT:(mt + 1) * DOUT], t3[0:rows], AF.Relu)
            if s >= SPC - 2:
                # last two sentences drain per row-block on the fast ring
                nc.sync.dma_start(
                    out=out_d[s, 0:rows, mt * DOUT:(mt + 1) * DOUT],
                    in_=ot[0:rows, mt * DOUT:(mt + 1) * DOUT])
                if mt == 1:
                    state.pop(s)
            elif mt == 1:
                nc.gpsimd.dma_start(out=out_d[s], in_=ot)
                state.pop(s)

        def emit_tail(s):
            # last unit fully fused: STT1 runs behind the out-direction
            # matmuls, then halved STT2/relu/DMA pipeline the drain.
            # zm is free after the last main tile's copy.
            st = state[s]
            mfb_t, zs, ot = st["mfb"], st["zs"], st["ot"]
            t1 = st.pop(("t1", 1))
            rows, H, d0 = 72, 180, 128
            ap_ = zmpool.tile([128, 2 * PSG], F32, tag="zm", name=f"ap{s}_tl")
            nc.tensor.matmul(ap_[0:rows, 0:361], lhsT=mfb_t[0:128, d0:d0 + rows],
                             rhs=zs[0:128, 0:361], start=True, stop=False)
            nc.tensor.matmul(ap_[0:rows, 0:361], lhsT=mfb_t[0:72, 200 + d0:200 + d0 + rows],
                             rhs=zs[0:72, B2:B2 + 361], start=False, stop=True)
            sgt = tpool.tile([128, 2], F32, tag="sgt", name=f"sg{s}_tl")
            nc.scalar.activation(sgt[0:rows, 0:1], ap_[0:rows, 360:361], AF.Sigmoid)
            t2 = tpool.tile([128, DOUT], BF16, tag="t2", name=f"t2{s}_tl")
            t3 = tpool.tile([128, DOUT], BF16, tag="t3", name=f"t3{s}_tl")
            for h in range(2):
                cs = slice(h * H, (h + 1) * H)
                nc.vector.scalar_tensor_tensor(
                    out=t2[0:rows, cs], in0=ap_[0:rows, h * H:h * H + H],
                    scalar=sgt[0:rows, 0:1], in1=t1[0:rows, cs],
                    op0=OP.mult, op1=OP.add)
            nc.tensor.matmul(ap_[0:rows, PSG:PSG + 361], lhsT=mfb_t[0:128, 400 + d0:400 + d0 + rows],
                             rhs=zs[0:128, GP:GP + 361], start=True, stop=False)
            nc.tensor.matmul(ap_[0:rows, PSG:PSG + 361], lhsT=mfb_t[0:72, 600 + d0:600 + d0 + rows],
                             rhs=zs[0:72, B2 + GP:B2 + GP + 361], start=False, stop=True)
            nc.scalar.activation(sgt[0:rows, 1:2], ap_[0:rows, PSG + 360:PSG + 361],
                                 AF.Sigmoid)
            for h in range(2):
                cs = slice(h * H, (h + 1) * H)
                nc.vector.scalar_tensor_tensor(
                    out=t3[0:rows, cs], in0=ap_[0:rows, PSG + h * H:PSG + h * H + H],
                    scalar=sgt[0:rows, 1:2], in1=t2[0:rows, cs],
                    op0=OP.mult, op1=OP.add)
                nc.scalar.activation(
                    ot[0:rows, DOUT + h * H:DOUT + (h + 1) * H],
                    t3[0:rows, cs], AF.Relu)
                nc.sync.dma_start(
                    out=out_d[s, 0:rows, DOUT + h * H:DOUT + (h + 1) * H],
                    in_=ot[0:rows, DOUT + h * H:DOUT + (h + 1) * H])
            state.pop(s)

        # ---- software pipeline: aggregation lags main by one sentence,
        # relu/output-DMA lag one more step ----
        for s in range(SPC):
            emit_main(s, 0)
            if s > 0:
                emit_agg(s - 1, 0)
            if s > 1:
                emit_relu(s - 2, 1)
            emit_main(s, 1)
            if s == SPC - 1:
                emit_agg(s - 1, 1)      # sentence 30 drains first: its
                emit_relu(s - 1, 0)     # output DMA is the long pole
                emit_agg(s, 0)
                emit_relu(s - 1, 1)
                emit_relu(s, 0)
                emit_tail(s)
            elif s > 0:
                emit_agg(s - 1, 1)
                emit_relu(s - 1, 0)

    nc.compile()
    return nc


def _get_compiled():
    global _compiled
    if _compiled is None:
        _compiled = _build()
    return _compiled


def kernel(gcn_in, adj_ind, adj_data, w_in, b_in, w_out, b_out, w_loop,
           w_gin, b_gin, w_gout, b_gout, w_gloop):
    from concourse.bass_utils import run_bass_kernel_spmd

    x = np.asarray(gcn_in, np.float32)           # [B, N, DIN]
    idx = np.asarray(adj_ind)[0]                 # [B, E, 2] int
    dat = np.asarray(adj_data, np.float32)[0]    # [B, E]

    # fused weight matrix with bias row
    wt = np.zeros((KA, 3 * GP), np.float32)
    for g, (w, gw, bias, gb) in enumerate([
        (w_in, w_gin, b_in, b_gin),
        (w_out, w_gout, b_out, b_gout),
        (w_loop, w_gloop, None, None),
    ]):
        wt[0:DIN, g * GP:g * GP + DOUT] = np.asarray(w, np.float32)
        wt[0:DIN, g * GP + DOUT] = np.asarray(gw, np.float32)[:, 0]
        if bias is not None:
            wt[DIN, g * GP:g * GP + DOUT] = np.asarray(bias, np.float32)[0]
            wt[DIN, g * GP + DOUT] = np.asarray(gb, np.float32)[0]
    wt = wt.astype(NBF16)

    # dense per-sentence adjacency matrices
    M = np.zeros((B, NN, NN), np.float32)
    bi = np.broadcast_to(np.arange(B)[:, None], idx.shape[:2])
    np.add.at(M, (bi, idx[:, :, 0].astype(np.int64), idx[:, :, 1].astype(np.int64)), dat)

    def chunked(mm):      # [SPC,200,200] -> [SPC,128,400]: two 128-row src chunks
        out = np.zeros((SPC, 128, 2 * NN), np.float32)
        out[:, :, 0:NN] = mm[:, 0:128, :]
        out[:, 0:72, NN:2 * NN] = mm[:, 128:200, :]
        return out

    nc = _get_compiled()
    in_maps = []
    for c in range(NCORES):
        xc = x[c * SPC:(c + 1) * SPC].reshape(ROWS, DIN)
        xt = np.empty((KA, ROWS), np.float32)
        xt[0:DIN] = xc.T
        xt[DIN] = 1.0
        mc = M[c * SPC:(c + 1) * SPC]
        mfb = np.concatenate(
            [chunked(mc), chunked(np.ascontiguousarray(mc.transpose(0, 2, 1)))],
            axis=2)
        xtw = np.concatenate([wt.astype(np.float32), xt], axis=1)
        in_maps.append({
            "xt": np.ascontiguousarray(xtw).astype(NBF16),
            "mfb": np.ascontiguousarray(mfb).astype(NBF16),
        })

    res = run_bass_kernel_spmd(nc, in_maps, core_ids=list(range(NCORES)))
    kernel.last_results = res
    out = np.empty((B, NN, DOUT), np.float32)
    for c in range(NCORES):
        oc = res.results[c]["out"].astype(np.float32)   # [SPC,128,720]
        oc_s = out[c * SPC:(c + 1) * SPC]               # [SPC,200,360]
        oc_s[:, 0:128, :] = oc[:, :, 0:DOUT]
        oc_s[:, 128:200, :] = oc[:, 0:72, DOUT:2 * DOUT]
    return out
